# revision 3
# baseline (speedup 1.0000x reference)
"""Multi-head self-attention prefill (B=2, S=2048, E=2048, H=16, D=128) on 8 trn2 cores.

Sharding: core c -> batch b = c//4, head-group g = c%4 (heads 4g..4g+3).
Each core computes q/k/v projections for its 4 heads (column shard of Wq/Wk/Wv),
causal attention with RoPE, and a partial output projection (row shard of Wo).
Host sums the 4 partials per batch (all-reduce equivalent) and stacks batches.

Attention is computed with transposed scores: s^T[k, q] = kT-tile^T @ qT, so the
exp() output (bf16) is directly the moving operand of the ctx matmul
ctx^T[d, q] = v^T @ exp(s^T) -- no PE transposes and no psum->sbuf copies in the
softmax chain. The softmax denominator z[q] comes from a ones-stationary matmul
over the same exp tiles (landing broadcast across partitions), and the 1/z scale
is applied once to ctx^T per (head, q-chunk) off the critical path.
"""
import sys
sys.path.insert(0, "/opt/trn_rl_repo")
import numpy as np

import concourse.bass as bass
import concourse.mybir as mybir
import concourse.tile as tile
from concourse import bacc
from concourse.bass import ds, ts
from concourse.masks import make_identity
from concourse.bass_utils import run_bass_kernel_spmd

S = 2048          # sequence length (per batch)
E = 2048          # embedding dim
H = 16            # total heads
D = 128           # head dim
HG = 4            # heads per core
DG = HG * D       # 512: per-core projection width
NE = E // 128     # 16 contraction chunks
NTB = 4           # token blocks of 512
TB = S // NTB     # 512
NTT = S // 128    # 16 token tiles of 128
NQC = 4           # q-chunks of 512
ROPE_BASE = 10000.0
MASK_VAL = -1e30

f32 = mybir.dt.float32
f32r = mybir.dt.float32r
bf16 = mybir.dt.bfloat16

_CACHE = {}


def build():
    nc = bacc.Bacc(None)
    x_in = nc.dram_tensor("x", [S, E], f32, kind="ExternalInput")
    wq_in = nc.dram_tensor("wq", [E, DG], f32, kind="ExternalInput")
    wk_in = nc.dram_tensor("wk", [E, DG], f32, kind="ExternalInput")
    wv_in = nc.dram_tensor("wv", [E, DG], f32, kind="ExternalInput")
    wo_in = nc.dram_tensor("wo", [DG, E], f32, kind="ExternalInput")
    cos_in = nc.dram_tensor("cosT", [128, S], f32, kind="ExternalInput")
    sin_in = nc.dram_tensor("sinT", [128, S], f32, kind="ExternalInput")
    out_d = nc.dram_tensor("out", [S, E], f32, kind="ExternalOutput")

    with tile.TileContext(nc) as tc:
        with tc.tile_pool(name="persist", bufs=1) as pp:
            # persistent across phases
            qT = [pp.tile([128, S], f32r, tag=f"qT{h}", name=f"qT{h}") for h in range(HG)]
            kT = [pp.tile([128, S], f32r, tag=f"kT{h}", name=f"kT{h}") for h in range(HG)]
            v_sb = [pp.tile([128, DG], bf16, tag=f"v{tt}", name=f"v{tt}") for tt in range(NTT)]
            ident = pp.tile([128, 128], f32r, tag="ident")
            identf = pp.tile([128, 128], f32, tag="identf")
            make_identity(nc, identf[:])
            nc.vector.tensor_copy(ident[:], identf[:])
            # transposed causal mask: maskT[k, q] = 0 where q >= k else MASK_VAL
            maskT = pp.tile([128, 128], f32, tag="maskT")
            nc.gpsimd.memset(maskT[:], 0.0)
            nc.gpsimd.affine_select(
                out=maskT[:], in_=maskT[:],
                compare_op=mybir.AluOpType.is_ge,
                fill=MASK_VAL, base=0,
                # iota(k, q) = -k + q; keep 0 where q - k >= 0, else MASK_VAL
                pattern=[[1, 128]], channel_multiplier=-1)
            ones_b = pp.tile([128, 128], bf16, tag="ones_b")
            nc.gpsimd.memset(ones_b[:], 1.0)

            # ---------------- Phase A: x^T, projections, RoPE ----------------
            with tc.tile_pool(name="phA", bufs=1) as pa, \
                 tc.tile_pool(name="phA2", bufs=2) as pa2, \
                 tc.tile_pool(name="psA", bufs=2, space="PSUM") as psA:
                cosT = pa.tile([128, S], f32r, tag="cos")
                nc.sync.dma_start(out=cosT[:], in_=cos_in[:].bitcast(f32r))
                sinT = pa.tile([128, S], f32r, tag="sin")
                nc.sync.dma_start(out=sinT[:], in_=sin_in[:].bitcast(f32r))

                for tb in range(NTB):
                    # load x rows [tb*512, +512) in two half-blocks, transpose to xT[e] [128, 512]
                    xTs = [pa.tile([128, TB], f32r, tag=f"xT{e}", name=f"xT{e}")
                           for e in range(NE)]
                    for half in range(2):
                        xh = []
                        for t2 in range(2):
                            xt = pa.tile([128, E], f32r, tag=f"x{t2}")
                            r0 = tb * TB + half * 256 + t2 * 128
                            nc.sync.dma_start(out=xt[:], in_=x_in[r0:r0 + 128, :].bitcast(f32r))
                            xh.append(xt)
                        for e in range(NE):
                            ps = psA.tile([128, 256], f32r, tag="ptx")
                            for t2 in range(2):
                                nc.tensor.transpose(ps[:, ts(t2, 128)], xh[t2][:, ts(e, 128)], ident[:])
                            (nc.vector.tensor_copy if e % 2 else nc.scalar.copy)(
                                xTs[e][:, ds(half * 256, 256)], ps[:])

                    # q/k projections: stationary = W chunk, moving = xT
                    for w_idx, w_in in ((0, wq_in), (1, wk_in)):
                        dstT = qT if w_idx == 0 else kT
                        for dhp in range(2):
                            wts = []
                            for e in range(NE):
                                wt = pa2.tile([128, 256], f32r, tag=f"w{e}")
                                nc.sync.dma_start(
                                    out=wt[:],
                                    in_=w_in[ts(e, 128), ds(dhp * 256, 256)].bitcast(f32r))
                                wts.append(wt)
                            for dh2 in range(2):
                                h = dhp * 2 + dh2
                                ps = psA.tile([128, TB], f32, tag="pqk")
                                for e in range(NE):
                                    nc.tensor.matmul(ps[:], wts[e][:, ts(dh2, 128)], xTs[e][:],
                                                     start=(e == 0), stop=(e == NE - 1))
                                sl = dstT[h][:, ts(tb, TB)]
                                # RoPE via staging tile (decoupled dep chains):
                                # sl = stage*cos + swap(stage)*sin
                                stg = pa2.tile([128, TB], f32r, tag="stage")
                                nc.scalar.copy(stg[:], ps[:])
                                swp = pa2.tile([128, TB], f32r, tag="swap")
                                nc.sync.dma_start(out=swp[0:64, :], in_=stg[64:128, :])
                                nc.sync.dma_start(out=swp[64:128, :], in_=stg[0:64, :])
                                nc.vector.tensor_mul(swp[:], swp[:], sinT[:, ts(tb, TB)])
                                nc.vector.tensor_mul(sl, stg[:], cosT[:, ts(tb, TB)])
                                nc.vector.tensor_add(sl, sl, swp[:])
                    # v projection: stationary = xT chunk, moving = Wv chunk
                    for dhp in range(2):
                        wts = []
                        for e in range(NE):
                            wt = pa2.tile([128, 256], f32r, tag=f"w{e}")
                            nc.sync.dma_start(
                                out=wt[:],
                                in_=wv_in[ts(e, 128), ds(dhp * 256, 256)].bitcast(f32r))
                            wts.append(wt)
                        for t4 in range(4):
                            tt = tb * 4 + t4
                            ps = psA.tile([128, 256], f32, tag="pv")
                            for e in range(NE):
                                nc.tensor.matmul(ps[:], xTs[e][:, ts(t4, 128)], wts[e][:],
                                                 start=(e == 0), stop=(e == NE - 1))
                            nc.scalar.copy(v_sb[tt][:, ds(dhp * 256, 256)], ps[:])

            # ---------------- Phase B: attention (transposed scores) + out-proj ----------------
            with tc.tile_pool(name="phB", bufs=1) as pb, \
                 tc.tile_pool(name="phB2", bufs=2) as pb2, \
                 tc.tile_pool(name="phB3", bufs=3) as pb3, \
                 tc.tile_pool(name="psS", bufs=3, space="PSUM") as psS, \
                 tc.tile_pool(name="psC", bufs=2, space="PSUM") as psC, \
                 tc.tile_pool(name="psZ", bufs=1, space="PSUM") as psZ, \
                 tc.tile_pool(name="psO", bufs=2, space="PSUM") as psO:
                for qc in range(NQC):
                    nkt = 4 * qc + 4        # k tiles needed for this q-chunk
                    q0 = qc * 512
                    ctxT = {}
                    for h in range(HG):
                        def issue_scores(kt):
                            """scores^T chunk [k-tile kt, q-chunk qc] -> exp -> bf16 aT."""
                            j = kt - 4 * qc   # >= 0 means diagonal straddle
                            c0 = max(0, j * 128)
                            w = 512 - c0
                            ps = psS.tile([128, 512], f32, tag="ps", name="ps")
                            nc.tensor.matmul(ps[:, ds(c0, w)], kT[h][:, ts(kt, 128)],
                                             qT[h][:, ds(q0 + c0, w)],
                                             start=True, stop=True)
                            if j >= 0:
                                nc.vector.tensor_add(ps[:, ds(c0, 128)],
                                                     ps[:, ds(c0, 128)], maskT[:])
                            at = pb3.tile([128, 512], bf16, tag="at", name="at")
                            nc.scalar.activation(at[:, ds(c0, w)], ps[:, ds(c0, w)],
                                                 mybir.ActivationFunctionType.Exp)
                            return at, c0, w

                        pc = psC.tile([128, 512], f32, tag="pc", name="pc")
                        pz = psZ.tile([128, 512], f32, tag="pz", name="pz")
                        pending = [issue_scores(0)]
                        if nkt > 1:
                            pending.append(issue_scores(1))
                        for kt in range(nkt):
                            if kt + 2 < nkt:
                                pending.append(issue_scores(kt + 2))
                            at, c0, w = pending[kt]
                            nc.tensor.matmul(pc[:, ds(c0, w)], v_sb[kt][:, ts(h, 128)],
                                             at[:, ds(c0, w)],
                                             start=(kt == 0), stop=(kt == nkt - 1),
                                             skip_group_check=True)
                            nc.tensor.matmul(pz[:, ds(c0, w)], ones_b[:],
                                             at[:, ds(c0, w)],
                                             start=(kt == 0), stop=(kt == nkt - 1),
                                             skip_group_check=True)
                        # normalize: ctxT = pc * (1/pz)
                        rz = pb2.tile([128, 512], f32, tag="rz", name="rz")
                        nc.vector.reciprocal(rz[:], pz[:])
                        ct = pb2.tile([128, 512], f32r, tag=f"ctxT{h}", name=f"ctxT{h}")
                        nc.vector.tensor_mul(ct[:], pc[:], rz[:])
                        ctxT[h] = ct

                    # --- output projection for this q-chunk (wo streamed) ---
                    for e4 in range(4):
                        wos = []
                        for h in range(HG):
                            w = pb2.tile([128, 512], f32r, tag=f"wo{h}", name=f"wo{h}")
                            nc.sync.dma_start(out=w[:],
                                              in_=wo_in[ts(h, 128), ts(e4, 512)].bitcast(f32r))
                            wos.append(w)
                        for t4 in range(4):
                            row0 = qc * 512 + t4 * 128
                            po = psO.tile([128, 512], f32, tag="po", name="po")
                            for h in range(HG):
                                nc.tensor.matmul(po[:], ctxT[h][:, ts(t4, 128)],
                                                 wos[h][:],
                                                 start=(h == 0), stop=(h == HG - 1))
                            ob = pb3.tile([128, 512], f32, tag="ob", name="ob", bufs=2)
                            nc.scalar.copy(ob[:], po[:])
                            nc.sync.dma_start(out=out_d[row0:row0 + 128, ts(e4, 512)],
                                              in_=ob[:])
    nc.finalize()
    return nc


def _host_tables():
    half = D // 2
    inv = 1.0 / (ROPE_BASE ** (np.arange(half, dtype=np.float64) * 2.0 / D))
    ang = np.arange(S, dtype=np.float64)[None, :] * inv[:, None]   # [64, S]
    cos = np.cos(ang).astype(np.float32)
    sin = np.sin(ang).astype(np.float32)
    cosT = np.concatenate([cos, cos], axis=0)                      # [128, S]
    sinT = np.concatenate([-sin, sin], axis=0)                     # [128, S]
    return cosT, sinT


def kernel(x, start_pos, Wq, Wk, Wv, Wo):
    x = np.asarray(x, dtype=np.float32)
    Wq = np.asarray(Wq, dtype=np.float32)
    Wk = np.asarray(Wk, dtype=np.float32)
    Wv = np.asarray(Wv, dtype=np.float32)
    Wo = np.asarray(Wo, dtype=np.float32)
    B = x.shape[0]
    assert x.shape == (B, S, E) and B == 2

    cosT, sinT = _host_tables()
    perm = np.concatenate([np.arange(0, D, 2), np.arange(1, D, 2)])
    scale = 1.0 / np.sqrt(D)

    in_maps = []
    for c in range(8):
        b, g = c // 4, c % 4
        cols = slice(DG * g, DG * g + DG)
        wq = (Wq[:, cols] * scale).astype(np.float32).reshape(E, HG, D)[:, :, perm].reshape(E, DG)
        wk = Wk[:, cols].reshape(E, HG, D)[:, :, perm].reshape(E, DG)
        wv = np.ascontiguousarray(Wv[:, cols])
        wo = np.ascontiguousarray(Wo[cols, :])
        in_maps.append({
            "x": np.ascontiguousarray(x[b]),
            "wq": np.ascontiguousarray(wq),
            "wk": np.ascontiguousarray(wk),
            "wv": wv,
            "wo": wo,
            "cosT": cosT,
            "sinT": sinT,
        })

    if "nc" not in _CACHE:
        _CACHE["nc"] = build()
    nc = _CACHE["nc"]
    _CACHE["in_maps"] = in_maps
    res = run_bass_kernel_spmd(nc, in_maps, list(range(8)))
    parts = [res.results[c]["out"] for c in range(8)]
    out = np.stack([
        parts[0] + parts[1] + parts[2] + parts[3],
        parts[4] + parts[5] + parts[6] + parts[7],
    ]).astype(np.float32)
    return out


# revision 4
# speedup vs baseline: 1.2087x; 1.2087x over previous
"""Multi-head self-attention prefill (B=2, S=2048, E=2048, H=16, D=128) on 8 trn2 cores.

Sharding: core c -> batch b = c//4, head-group g = c%4 (heads 4g..4g+3).
Each core computes q/k/v projections for its 4 heads (column shard of Wq/Wk/Wv),
causal attention with RoPE, and a partial output projection (row shard of Wo).
Host sums the 4 partials per batch (all-reduce equivalent) and stacks batches.

Attention is computed with transposed scores: s^T[k, q] = kT-tile^T @ qT, so the
exp() output (bf16) is directly the moving operand of the ctx matmul
ctx^T[d, q] = v^T @ exp(s^T) -- no PE transposes and no psum->sbuf copies in the
softmax chain. Causal masking zeroes the exp output's lower triangle on the
(otherwise idle) gpsimd engine. The softmax denominator z[q] comes from a
ones-stationary matmul over the same exp tiles (landing broadcast across
partitions); the 1/z scale is applied once to ctx^T per (head, q-chunk).
"""
import sys
sys.path.insert(0, "/opt/trn_rl_repo")
import numpy as np

import concourse.bass as bass
import concourse.mybir as mybir
import concourse.tile as tile
from concourse import bacc
from concourse.bass import ds, ts
from concourse.masks import make_identity
from concourse.bass_utils import run_bass_kernel_spmd

S = 2048          # sequence length (per batch)
E = 2048          # embedding dim
H = 16            # total heads
D = 128           # head dim
HG = 4            # heads per core
DG = HG * D       # 512: per-core projection width
NE = E // 128     # 16 contraction chunks
NTB = 4           # token blocks of 512
TB = S // NTB     # 512
NTT = S // 128    # 16 token tiles of 128
NQC = 4           # q-chunks of 512
ROPE_BASE = 10000.0

f32 = mybir.dt.float32
f32r = mybir.dt.float32r
bf16 = mybir.dt.bfloat16

_CACHE = {}


def build():
    nc = bacc.Bacc(None)
    x_in = nc.dram_tensor("x", [S, E], f32, kind="ExternalInput")
    wq_in = nc.dram_tensor("wq", [E, DG], f32, kind="ExternalInput")
    wk_in = nc.dram_tensor("wk", [E, DG], f32, kind="ExternalInput")
    wv_in = nc.dram_tensor("wv", [E, DG], f32, kind="ExternalInput")
    wo_in = nc.dram_tensor("wo", [DG, E], f32, kind="ExternalInput")
    cos_in = nc.dram_tensor("cosT", [128, S], f32, kind="ExternalInput")
    sin_in = nc.dram_tensor("sinT", [128, S], f32, kind="ExternalInput")
    out_d = nc.dram_tensor("out", [S, E], f32, kind="ExternalOutput")

    with tile.TileContext(nc) as tc:
        with tc.tile_pool(name="persist", bufs=1) as pp:
            # persistent across phases
            qT = [pp.tile([128, S], f32r, tag=f"qT{h}", name=f"qT{h}") for h in range(HG)]
            kT = [pp.tile([128, S], f32r, tag=f"kT{h}", name=f"kT{h}") for h in range(HG)]
            v_sb = [pp.tile([128, DG], bf16, tag=f"v{tt}", name=f"v{tt}") for tt in range(NTT)]
            ident = pp.tile([128, 128], f32r, tag="ident")
            identf = pp.tile([128, 128], f32, tag="identf")
            make_identity(nc, identf[:])
            nc.vector.tensor_copy(ident[:], identf[:])
            ones_b = pp.tile([128, 128], bf16, tag="ones_b")
            nc.gpsimd.memset(ones_b[:], 1.0)

            # ---------------- Phase A: x^T, projections, RoPE ----------------
            with tc.tile_pool(name="phA", bufs=1) as pa, \
                 tc.tile_pool(name="phA2", bufs=2) as pa2, \
                 tc.tile_pool(name="psA", bufs=2, space="PSUM") as psA:
                cosT = pa.tile([128, S], f32r, tag="cos")
                nc.sync.dma_start(out=cosT[:], in_=cos_in[:].bitcast(f32r))
                sinT = pa.tile([128, S], f32r, tag="sin")
                nc.sync.dma_start(out=sinT[:], in_=sin_in[:].bitcast(f32r))

                for tb in range(NTB):
                    # load x rows [tb*512, +512) in two half-blocks (double-
                    # buffered so DMA prefetches ahead), transpose to xT[e]
                    xTs = [pa.tile([128, TB], f32r, tag=f"xT{e}", name=f"xT{e}")
                           for e in range(NE)]
                    for half in range(2):
                        xh = []
                        for t2 in range(2):
                            xt = pa2.tile([128, E], f32r, tag=f"x{t2}", bufs=2)
                            r0 = tb * TB + half * 256 + t2 * 128
                            nc.sync.dma_start(out=xt[:], in_=x_in[r0:r0 + 128, :].bitcast(f32r))
                            xh.append(xt)
                        for e in range(NE):
                            ps = psA.tile([128, 256], f32r, tag="ptx")
                            for t2 in range(2):
                                nc.tensor.transpose(ps[:, ts(t2, 128)], xh[t2][:, ts(e, 128)], ident[:])
                            (nc.vector.tensor_copy if e % 2 else nc.scalar.copy)(
                                xTs[e][:, ds(half * 256, 256)], ps[:])

                    # q/k projections: stationary = W chunk, moving = xT
                    for w_idx, w_in in ((0, wq_in), (1, wk_in)):
                        dstT = qT if w_idx == 0 else kT
                        wts = []
                        for e in range(NE):
                            wt = pa2.tile([128, DG], f32r, tag=f"w{e}", bufs=1)
                            nc.sync.dma_start(out=wt[:],
                                              in_=w_in[ts(e, 128), :].bitcast(f32r))
                            wts.append(wt)
                        for h in range(HG):
                            ps = psA.tile([128, TB], f32, tag="pqk")
                            for e in range(NE):
                                nc.tensor.matmul(ps[:], wts[e][:, ts(h, 128)], xTs[e][:],
                                                 start=(e == 0), stop=(e == NE - 1))
                            sl = dstT[h][:, ts(tb, TB)]
                            # RoPE via staging tile (decoupled dep chains):
                            # sl = stage*cos + swap(stage)*sin
                            stg = pa2.tile([128, TB], f32r, tag="stage")
                            nc.scalar.copy(stg[:], ps[:])
                            swp = pa2.tile([128, TB], f32r, tag="swap")
                            nc.sync.dma_start(out=swp[0:64, :], in_=stg[64:128, :])
                            nc.sync.dma_start(out=swp[64:128, :], in_=stg[0:64, :])
                            nc.vector.tensor_mul(swp[:], swp[:], sinT[:, ts(tb, TB)])
                            nc.vector.tensor_mul(sl, stg[:], cosT[:, ts(tb, TB)])
                            nc.vector.tensor_add(sl, sl, swp[:])
                    # v projection: stationary = xT chunk, moving = Wv chunk
                    wts = []
                    for e in range(NE):
                        wt = pa2.tile([128, DG], f32r, tag=f"w{e}", bufs=1)
                        nc.sync.dma_start(out=wt[:],
                                          in_=wv_in[ts(e, 128), :].bitcast(f32r))
                        wts.append(wt)
                    for t4 in range(4):
                        tt = tb * 4 + t4
                        ps = psA.tile([128, DG], f32, tag="pv")
                        for e in range(NE):
                            nc.tensor.matmul(ps[:], xTs[e][:, ts(t4, 128)], wts[e][:],
                                             start=(e == 0), stop=(e == NE - 1))
                        nc.scalar.copy(v_sb[tt][:], ps[:])

            # ---------------- Phase B: attention (transposed scores) + out-proj ----------------
            with tc.tile_pool(name="phB", bufs=1) as pb, \
                 tc.tile_pool(name="phB2", bufs=2) as pb2, \
                 tc.tile_pool(name="phB3", bufs=3) as pb3, \
                 tc.tile_pool(name="psS", bufs=3, space="PSUM") as psS, \
                 tc.tile_pool(name="psC", bufs=2, space="PSUM") as psC, \
                 tc.tile_pool(name="psZ", bufs=1, space="PSUM") as psZ, \
                 tc.tile_pool(name="psO", bufs=2, space="PSUM") as psO:
                # wo resident in SBUF for the whole phase (loads overlap qc=0)
                wo_sb = [pb.tile([128, E], f32r, tag=f"wosb{h}", name=f"wosb{h}")
                         for h in range(HG)]
                for h in range(HG):
                    nc.sync.dma_start(out=wo_sb[h][:],
                                      in_=wo_in[ts(h, 128), :].bitcast(f32r))

                for qc in range(NQC):
                    nkt = 4 * qc + 4        # k tiles needed for this q-chunk
                    q0 = qc * 512
                    ctxT = {}
                    for h in range(HG):
                        def issue_scores(kt):
                            """scores^T chunk [k-tile kt, q-chunk qc] -> exp -> bf16 aT."""
                            j = kt - 4 * qc   # >= 0 means diagonal straddle
                            c0 = max(0, j * 128)
                            w = 512 - c0
                            ps = psS.tile([128, 512], f32, tag="ps", name="ps")
                            nc.tensor.matmul(ps[:, ds(c0, w)], kT[h][:, ts(kt, 128)],
                                             qT[h][:, ds(q0 + c0, w)],
                                             start=True, stop=True)
                            at = pb3.tile([128, 512], bf16, tag="at", name="at")
                            nc.scalar.activation(at[:, ds(c0, w)], ps[:, ds(c0, w)],
                                                 mybir.ActivationFunctionType.Exp)
                            if j >= 0:
                                # causal: zero the strictly-lower triangle
                                # (q < k) of the diagonal 128x128 block
                                nc.gpsimd.affine_select(
                                    out=at[:, ds(c0, 128)], in_=at[:, ds(c0, 128)],
                                    compare_op=mybir.AluOpType.is_ge,
                                    fill=0.0, base=0,
                                    pattern=[[1, 128]], channel_multiplier=-1)
                            return at, c0, w

                        pc = psC.tile([128, 512], f32, tag="pc", name="pc")
                        pz = psZ.tile([128, 512], f32, tag="pz", name="pz")
                        pending = [issue_scores(0)]
                        if nkt > 1:
                            pending.append(issue_scores(1))
                        for kt in range(nkt):
                            if kt + 2 < nkt:
                                pending.append(issue_scores(kt + 2))
                            at, c0, w = pending[kt]
                            nc.tensor.matmul(pc[:, ds(c0, w)], v_sb[kt][:, ts(h, 128)],
                                             at[:, ds(c0, w)],
                                             start=(kt == 0), stop=(kt == nkt - 1),
                                             skip_group_check=True)
                            nc.tensor.matmul(pz[:, ds(c0, w)], ones_b[:],
                                             at[:, ds(c0, w)],
                                             start=(kt == 0), stop=(kt == nkt - 1),
                                             skip_group_check=True)
                        # normalize: ctxT = pc * (1/pz)
                        rz = pb2.tile([128, 512], f32, tag="rz", name="rz")
                        nc.vector.reciprocal_approx_fast(out=rz[:], in_=pz[:])
                        ct = pb2.tile([128, 512], f32r, tag=f"ctxT{h}", name=f"ctxT{h}")
                        nc.vector.tensor_mul(ct[:], pc[:], rz[:])
                        ctxT[h] = ct

                    # --- output projection for this q-chunk ---
                    for e4 in range(4):
                        for t4 in range(4):
                            row0 = qc * 512 + t4 * 128
                            po = psO.tile([128, 512], f32, tag="po", name="po")
                            for h in range(HG):
                                nc.tensor.matmul(po[:], ctxT[h][:, ts(t4, 128)],
                                                 wo_sb[h][:, ts(e4, 512)],
                                                 start=(h == 0), stop=(h == HG - 1))
                            ob = pb3.tile([128, 512], f32, tag="ob", name="ob", bufs=2)
                            nc.scalar.copy(ob[:], po[:])
                            nc.sync.dma_start(out=out_d[row0:row0 + 128, ts(e4, 512)],
                                              in_=ob[:])
    nc.finalize()
    return nc


def _host_tables():
    half = D // 2
    inv = 1.0 / (ROPE_BASE ** (np.arange(half, dtype=np.float64) * 2.0 / D))
    ang = np.arange(S, dtype=np.float64)[None, :] * inv[:, None]   # [64, S]
    cos = np.cos(ang).astype(np.float32)
    sin = np.sin(ang).astype(np.float32)
    cosT = np.concatenate([cos, cos], axis=0)                      # [128, S]
    sinT = np.concatenate([-sin, sin], axis=0)                     # [128, S]
    return cosT, sinT


def kernel(x, start_pos, Wq, Wk, Wv, Wo):
    x = np.asarray(x, dtype=np.float32)
    Wq = np.asarray(Wq, dtype=np.float32)
    Wk = np.asarray(Wk, dtype=np.float32)
    Wv = np.asarray(Wv, dtype=np.float32)
    Wo = np.asarray(Wo, dtype=np.float32)
    B = x.shape[0]
    assert x.shape == (B, S, E) and B == 2

    cosT, sinT = _host_tables()
    perm = np.concatenate([np.arange(0, D, 2), np.arange(1, D, 2)])
    scale = 1.0 / np.sqrt(D)

    in_maps = []
    for c in range(8):
        b, g = c // 4, c % 4
        cols = slice(DG * g, DG * g + DG)
        wq = (Wq[:, cols] * scale).astype(np.float32).reshape(E, HG, D)[:, :, perm].reshape(E, DG)
        wk = Wk[:, cols].reshape(E, HG, D)[:, :, perm].reshape(E, DG)
        wv = np.ascontiguousarray(Wv[:, cols])
        wo = np.ascontiguousarray(Wo[cols, :])
        in_maps.append({
            "x": np.ascontiguousarray(x[b]),
            "wq": np.ascontiguousarray(wq),
            "wk": np.ascontiguousarray(wk),
            "wv": wv,
            "wo": wo,
            "cosT": cosT,
            "sinT": sinT,
        })

    if "nc" not in _CACHE:
        _CACHE["nc"] = build()
    nc = _CACHE["nc"]
    _CACHE["in_maps"] = in_maps
    res = run_bass_kernel_spmd(nc, in_maps, list(range(8)))
    parts = [res.results[c]["out"] for c in range(8)]
    out = np.stack([
        parts[0] + parts[1] + parts[2] + parts[3],
        parts[4] + parts[5] + parts[6] + parts[7],
    ]).astype(np.float32)
    return out


# revision 6
# speedup vs baseline: 1.4960x; 1.2377x over previous
"""Multi-head self-attention prefill (B=2, S=2048, E=2048, H=16, D=128) on 8 trn2 cores.

Sharding: core c -> batch b = c//4, head-group g = c%4 (heads 4g..4g+3).
Each core computes q/k/v projections for its 4 heads (column shard of Wq/Wk/Wv),
causal attention with RoPE, and a partial output projection (row shard of Wo).
Host sums the 4 partials per batch (all-reduce equivalent) and stacks batches.

Phase A: x arrives bf16 and is transposed by the DMA XBAR directly into
full-sequence xT tiles; Wq/Wk/Wv (bf16, host-cast) then stream through exactly
once, with each stationary load amortized over 4 wide matmuls. RoPE runs in
f32; qT/kT stay f32r.

Phase B: attention with transposed scores: s^T[k, q] = kT-tile^T @ qT, so the
exp() output (bf16) is directly the moving operand of the ctx matmul
ctx^T[d, q] = v^T @ exp(s^T) -- no PE transposes and no psum->sbuf copies in
the softmax chain. Causal masking zeroes the exp output's lower triangle on the
(otherwise idle) gpsimd engine. The softmax denominator z[q] comes from a
ones-stationary matmul over the same exp tiles (landing broadcast across
partitions); the 1/z scale is applied once to ctx^T per (head, q-chunk).
"""
import sys
sys.path.insert(0, "/opt/trn_rl_repo")
import numpy as np
import ml_dtypes

import concourse.bass as bass
import concourse.mybir as mybir
import concourse.tile as tile
from concourse import bacc
from concourse.bass import ds, ts
from concourse.bass_utils import run_bass_kernel_spmd

S = 2048          # sequence length (per batch)
E = 2048          # embedding dim
H = 16            # total heads
D = 128           # head dim
HG = 4            # heads per core
DG = HG * D       # 512: per-core projection width
NE = E // 128     # 16 contraction chunks
NTT = S // 128    # 16 token tiles of 128
NSC = 4           # sequence chunks of 512
NQC = 4           # q-chunks of 512
ROPE_BASE = 10000.0

f32 = mybir.dt.float32
f32r = mybir.dt.float32r
bf16 = mybir.dt.bfloat16

_CACHE = {}


def build():
    nc = bacc.Bacc(None)
    x_in = nc.dram_tensor("x", [S, E], bf16, kind="ExternalInput")
    wq_in = nc.dram_tensor("wq", [E, DG], bf16, kind="ExternalInput")
    wk_in = nc.dram_tensor("wk", [E, DG], bf16, kind="ExternalInput")
    wv_in = nc.dram_tensor("wv", [E, DG], bf16, kind="ExternalInput")
    wo_in = nc.dram_tensor("wo", [DG, E], f32, kind="ExternalInput")
    cos_in = nc.dram_tensor("cosT", [128, S], f32, kind="ExternalInput")
    sin_in = nc.dram_tensor("sinT", [128, S], f32, kind="ExternalInput")
    out_d = nc.dram_tensor("out", [S, E], f32, kind="ExternalOutput")

    with tile.TileContext(nc) as tc:
        with tc.tile_pool(name="persist", bufs=1) as pp:
            # persistent across phases
            qT = [pp.tile([128, S], f32r, tag=f"qT{h}", name=f"qT{h}") for h in range(HG)]
            kT = [pp.tile([128, S], f32r, tag=f"kT{h}", name=f"kT{h}") for h in range(HG)]
            v_sb = [pp.tile([128, DG], bf16, tag=f"v{tt}", name=f"v{tt}") for tt in range(NTT)]
            ones_b = pp.tile([128, 128], bf16, tag="ones_b")
            nc.gpsimd.memset(ones_b[:], 1.0)

            # ---------------- Phase A: x^T (DMA xbar), projections, RoPE ----------------
            with tc.tile_pool(name="phA", bufs=1) as pa, \
                 tc.tile_pool(name="phA2", bufs=2) as pa2, \
                 tc.tile_pool(name="psA", bufs=1, space="PSUM") as psA:
                # x^T resident for all of phase A, transposed on load
                xTs = [pa.tile([128, S], bf16, tag=f"xT{e}", name=f"xT{e}")
                       for e in range(NE)]
                for e in range(NE):
                    nc.sync.dma_start_transpose(out=xTs[e][:], in_=x_in[:, ts(e, 128)])
                cosT = pa.tile([128, S], f32r, tag="cos")
                nc.sync.dma_start(out=cosT[:], in_=cos_in[:].bitcast(f32r))
                sinT = pa.tile([128, S], f32r, tag="sin")
                nc.sync.dma_start(out=sinT[:], in_=sin_in[:].bitcast(f32r))

                # q/k projections: stationary = W chunk (reused over 4 wide
                # moving matmuls), psum accumulation over e
                for w_idx, w_in in ((0, wq_in), (1, wk_in)):
                    dstT = qT if w_idx == 0 else kT
                    wts = []
                    for e in range(NE):
                        wt = pa2.tile([128, DG], bf16, tag=f"w{e}", bufs=2)
                        nc.sync.dma_start(out=wt[:], in_=w_in[ts(e, 128), :])
                        wts.append(wt)
                    for h in range(HG):
                        pss = [psA.tile([128, 512], f32, tag=f"pqk{sc}", name=f"pqk{sc}")
                               for sc in range(NSC)]
                        for e in range(NE):
                            for sc in range(NSC):
                                nc.tensor.matmul(pss[sc][:], wts[e][:, ts(h, 128)],
                                                 xTs[e][:, ts(sc, 512)],
                                                 start=(e == 0), stop=(e == NE - 1))
                        for sc in range(NSC):
                            sl = dstT[h][:, ts(sc, 512)]
                            # RoPE: sl = stage*cos + swap(stage)*sin
                            stg = pa2.tile([128, 512], f32r, tag="stage")
                            nc.scalar.copy(stg[:], pss[sc][:])
                            swp = pa2.tile([128, 512], f32r, tag="swap")
                            nc.sync.dma_start(out=swp[0:64, :], in_=stg[64:128, :])
                            nc.sync.dma_start(out=swp[64:128, :], in_=stg[0:64, :])
                            nc.vector.tensor_mul(swp[:], swp[:], sinT[:, ts(sc, 512)])
                            nc.vector.tensor_mul(sl, stg[:], cosT[:, ts(sc, 512)])
                            nc.vector.tensor_add(sl, sl, swp[:])
                # v projection: stationary = xT chunk, moving = Wv chunk
                wts = []
                for e in range(NE):
                    wt = pa2.tile([128, DG], bf16, tag=f"w{e}", bufs=2)
                    nc.sync.dma_start(out=wt[:], in_=wv_in[ts(e, 128), :])
                    wts.append(wt)
                for tt in range(NTT):
                    psv = psA.tile([128, DG], f32, tag="pv", name="pv", bufs=2)
                    for e in range(NE):
                        nc.tensor.matmul(psv[:], xTs[e][:, ts(tt, 128)], wts[e][:],
                                         start=(e == 0), stop=(e == NE - 1))
                    nc.scalar.copy(v_sb[tt][:], psv[:])

            # ---------------- Phase B: attention (transposed scores) + out-proj ----------------
            with tc.tile_pool(name="phB", bufs=1) as pb, \
                 tc.tile_pool(name="phB2", bufs=2) as pb2, \
                 tc.tile_pool(name="phB3", bufs=3) as pb3, \
                 tc.tile_pool(name="psS", bufs=3, space="PSUM") as psS, \
                 tc.tile_pool(name="psC", bufs=2, space="PSUM") as psC, \
                 tc.tile_pool(name="psZ", bufs=1, space="PSUM") as psZ, \
                 tc.tile_pool(name="psO", bufs=2, space="PSUM") as psO:
                # wo resident in SBUF for the whole phase (loads overlap qc=0)
                wo_sb = [pb.tile([128, E], f32r, tag=f"wosb{h}", name=f"wosb{h}")
                         for h in range(HG)]
                for h in range(HG):
                    nc.sync.dma_start(out=wo_sb[h][:],
                                      in_=wo_in[ts(h, 128), :].bitcast(f32r))

                for qc in range(NQC):
                    nkt = 4 * qc + 4        # k tiles needed for this q-chunk
                    q0 = qc * 512
                    ctxT = {}
                    for h in range(HG):
                        def issue_scores(kt):
                            """scores^T chunk [k-tile kt, q-chunk qc] -> exp -> bf16 aT."""
                            j = kt - 4 * qc   # >= 0 means diagonal straddle
                            c0 = max(0, j * 128)
                            w = 512 - c0
                            ps = psS.tile([128, 512], f32, tag="ps", name="ps")
                            nc.tensor.matmul(ps[:, ds(c0, w)], kT[h][:, ts(kt, 128)],
                                             qT[h][:, ds(q0 + c0, w)],
                                             start=True, stop=True)
                            at = pb3.tile([128, 512], bf16, tag="at", name="at")
                            nc.scalar.activation(at[:, ds(c0, w)], ps[:, ds(c0, w)],
                                                 mybir.ActivationFunctionType.Exp)
                            if j >= 0:
                                # causal: zero the strictly-lower triangle
                                # (q < k) of the diagonal 128x128 block
                                nc.gpsimd.affine_select(
                                    out=at[:, ds(c0, 128)], in_=at[:, ds(c0, 128)],
                                    compare_op=mybir.AluOpType.is_ge,
                                    fill=0.0, base=0,
                                    pattern=[[1, 128]], channel_multiplier=-1)
                            return at, c0, w

                        pc = psC.tile([128, 512], f32, tag="pc", name="pc")
                        pz = psZ.tile([128, 512], f32, tag="pz", name="pz")
                        pending = [issue_scores(0)]
                        if nkt > 1:
                            pending.append(issue_scores(1))
                        for kt in range(nkt):
                            if kt + 2 < nkt:
                                pending.append(issue_scores(kt + 2))
                            at, c0, w = pending[kt]
                            nc.tensor.matmul(pc[:, ds(c0, w)], v_sb[kt][:, ts(h, 128)],
                                             at[:, ds(c0, w)],
                                             start=(kt == 0), stop=(kt == nkt - 1),
                                             skip_group_check=True)
                            nc.tensor.matmul(pz[:, ds(c0, w)], ones_b[:],
                                             at[:, ds(c0, w)],
                                             start=(kt == 0), stop=(kt == nkt - 1),
                                             skip_group_check=True)
                        # normalize: ctxT = pc * (1/pz)
                        rz = pb2.tile([128, 512], f32, tag="rz", name="rz")
                        nc.vector.reciprocal_approx_fast(out=rz[:], in_=pz[:])
                        ct = pb2.tile([128, 512], f32r, tag=f"ctxT{h}", name=f"ctxT{h}")
                        nc.vector.tensor_mul(ct[:], pc[:], rz[:])
                        ctxT[h] = ct

                    # --- output projection for this q-chunk ---
                    for e4 in range(4):
                        for t4 in range(4):
                            row0 = qc * 512 + t4 * 128
                            po = psO.tile([128, 512], f32, tag="po", name="po")
                            for h in range(HG):
                                nc.tensor.matmul(po[:], ctxT[h][:, ts(t4, 128)],
                                                 wo_sb[h][:, ts(e4, 512)],
                                                 start=(h == 0), stop=(h == HG - 1))
                            ob = pb3.tile([128, 512], f32, tag="ob", name="ob", bufs=2)
                            nc.vector.tensor_copy(ob[:], po[:])
                            nc.sync.dma_start(out=out_d[row0:row0 + 128, ts(e4, 512)],
                                              in_=ob[:])
    nc.finalize()
    return nc


def _host_tables():
    half = D // 2
    inv = 1.0 / (ROPE_BASE ** (np.arange(half, dtype=np.float64) * 2.0 / D))
    ang = np.arange(S, dtype=np.float64)[None, :] * inv[:, None]   # [64, S]
    cos = np.cos(ang).astype(np.float32)
    sin = np.sin(ang).astype(np.float32)
    cosT = np.concatenate([cos, cos], axis=0)                      # [128, S]
    sinT = np.concatenate([-sin, sin], axis=0)                     # [128, S]
    return cosT, sinT


def kernel(x, start_pos, Wq, Wk, Wv, Wo):
    x = np.asarray(x, dtype=np.float32)
    Wq = np.asarray(Wq, dtype=np.float32)
    Wk = np.asarray(Wk, dtype=np.float32)
    Wv = np.asarray(Wv, dtype=np.float32)
    Wo = np.asarray(Wo, dtype=np.float32)
    B = x.shape[0]
    assert x.shape == (B, S, E) and B == 2

    cosT, sinT = _host_tables()
    perm = np.concatenate([np.arange(0, D, 2), np.arange(1, D, 2)])
    scale = 1.0 / np.sqrt(D)
    bf = ml_dtypes.bfloat16

    in_maps = []
    for c in range(8):
        b, g = c // 4, c % 4
        cols = slice(DG * g, DG * g + DG)
        wq = (Wq[:, cols] * scale).astype(np.float32).reshape(E, HG, D)[:, :, perm].reshape(E, DG)
        wk = Wk[:, cols].reshape(E, HG, D)[:, :, perm].reshape(E, DG)
        wv = Wv[:, cols]
        wo = np.ascontiguousarray(Wo[cols, :])
        in_maps.append({
            "x": np.ascontiguousarray(x[b].astype(bf)),
            "wq": np.ascontiguousarray(wq.astype(bf)),
            "wk": np.ascontiguousarray(wk.astype(bf)),
            "wv": np.ascontiguousarray(wv.astype(bf)),
            "wo": wo,
            "cosT": cosT,
            "sinT": sinT,
        })

    if "nc" not in _CACHE:
        _CACHE["nc"] = build()
    nc = _CACHE["nc"]
    _CACHE["in_maps"] = in_maps
    res = run_bass_kernel_spmd(nc, in_maps, list(range(8)))
    parts = [res.results[c]["out"] for c in range(8)]
    out = np.stack([
        parts[0] + parts[1] + parts[2] + parts[3],
        parts[4] + parts[5] + parts[6] + parts[7],
    ]).astype(np.float32)
    return out


# revision 10
# speedup vs baseline: 1.5175x; 1.0143x over previous
"""Multi-head self-attention prefill (B=2, S=2048, E=2048, H=16, D=128) on 8 trn2 cores.

Sharding: core c -> batch b = c//4, head-group g = c%4 (heads 4g..4g+3).
Each core computes q/k/v projections for its 4 heads (column shard of Wq/Wk/Wv),
causal attention with RoPE, and a partial output projection (row shard of Wo).
Host sums the 4 partials per batch (all-reduce equivalent) and stacks batches.

Phase A: x arrives already transposed (and bf16-cast) from the host as xT[E,S];
it streams in sequence-chunk-major order so the first projection group starts
after ~2MB. Wq/Wk/Wv (bf16, host-cast) are SBUF-resident (24KB/partition) and
each is read exactly once from DRAM. RoPE runs in f32; qT/kT stay f32r.

Phase B: attention with transposed scores: s^T[k, q] = kT-tile^T @ qT, so the
exp() output (bf16) is directly the moving operand of the ctx matmul
ctx^T[d, q] = v^T @ exp(s^T) -- no PE transposes and no psum->sbuf copies in
the softmax chain. Causal masking zeroes the exp output's lower triangle on the
(otherwise idle) gpsimd engine. The softmax denominator z[q] comes from a
ones-stationary matmul over the same exp tiles (landing broadcast across
partitions); the 1/z scale is applied once to ctx^T per (head, q-chunk).
"""
import sys
sys.path.insert(0, "/opt/trn_rl_repo")
import numpy as np
import ml_dtypes

import concourse.bass as bass
import concourse.mybir as mybir
import concourse.tile as tile
from concourse import bacc
from concourse.bass import ds, ts
from concourse.bass_utils import run_bass_kernel_spmd

S = 2048          # sequence length (per batch)
E = 2048          # embedding dim
H = 16            # total heads
D = 128           # head dim
HG = 4            # heads per core
DG = HG * D       # 512: per-core projection width
NE = E // 128     # 16 contraction chunks
NTT = S // 128    # 16 token tiles of 128
NSC = 4           # sequence chunks of 512
NQC = 4           # q-chunks of 512
ROPE_BASE = 10000.0

f32 = mybir.dt.float32
f32r = mybir.dt.float32r
bf16 = mybir.dt.bfloat16

_CACHE = {}


def build():
    nc = bacc.Bacc(None)
    xt_in = nc.dram_tensor("xt", [E, S], bf16, kind="ExternalInput")
    wq_in = nc.dram_tensor("wq", [E, DG], bf16, kind="ExternalInput")
    wk_in = nc.dram_tensor("wk", [E, DG], bf16, kind="ExternalInput")
    wv_in = nc.dram_tensor("wv", [E, DG], bf16, kind="ExternalInput")
    wo_in = nc.dram_tensor("wo", [DG, E], f32, kind="ExternalInput")
    cos_in = nc.dram_tensor("cosT", [128, S], f32, kind="ExternalInput")
    sin_in = nc.dram_tensor("sinT", [128, S], f32, kind="ExternalInput")
    out_d = nc.dram_tensor("out", [S, E], f32, kind="ExternalOutput")

    with tile.TileContext(nc) as tc:
        with tc.tile_pool(name="persist", bufs=1) as pp:
            # persistent across phases
            qT = [pp.tile([128, S], f32r, tag=f"qT{h}", name=f"qT{h}") for h in range(HG)]
            kT = [pp.tile([128, S], f32r, tag=f"kT{h}", name=f"kT{h}") for h in range(HG)]
            v_sb = [pp.tile([128, DG], bf16, tag=f"v{tt}", name=f"v{tt}") for tt in range(NTT)]
            ones_b = pp.tile([128, 128], bf16, tag="ones_b")
            nc.gpsimd.memset(ones_b[:], 1.0)

            # ---------------- Phase A: projections + RoPE ----------------
            with tc.tile_pool(name="phA", bufs=1) as pa, \
                 tc.tile_pool(name="phA2", bufs=2) as pa2, \
                 tc.tile_pool(name="psA", bufs=2, space="PSUM") as psA:
                def load_x_chunk(sc):
                    """x^T tiles for one 512-token sequence chunk (bufs=2 so the
                    next chunk's loads prefetch during this chunk's compute)."""
                    tiles = []
                    for e in range(NE):
                        xt = pa2.tile([128, 512], bf16, tag=f"xT{e}", name=f"xT{e}")
                        nc.sync.dma_start(out=xt[:],
                                          in_=xt_in[ts(e, 128), ts(sc, 512)])
                        tiles.append(xt)
                    return tiles

                xcur = load_x_chunk(0)
                # W + tables queue behind the first x chunk
                wqs, wks, wvs = [], [], []
                for tag, w_in, lst in (("wq", wq_in, wqs), ("wk", wk_in, wks),
                                       ("wv", wv_in, wvs)):
                    for e in range(NE):
                        wt = pa.tile([128, DG], bf16, tag=f"{tag}{e}",
                                     name=f"{tag}{e}")
                        nc.sync.dma_start(out=wt[:], in_=w_in[ts(e, 128), :])
                        lst.append(wt)
                cosT = pa.tile([128, S], f32r, tag="cos")
                nc.sync.dma_start(out=cosT[:], in_=cos_in[:].bitcast(f32r))
                sinT = pa.tile([128, S], f32r, tag="sin")
                nc.sync.dma_start(out=sinT[:], in_=sin_in[:].bitcast(f32r))

                for sc in range(NSC):
                    xTs = xcur
                    if sc + 1 < NSC:
                        xcur = load_x_chunk(sc + 1)
                    # q/k projections for this sequence chunk
                    for wts, dstT in ((wqs, qT), (wks, kT)):
                        for h in range(HG):
                            ps = psA.tile([128, 512], f32, tag="pp", name="pp")
                            for e in range(NE):
                                nc.tensor.matmul(ps[:], wts[e][:, ts(h, 128)],
                                                 xTs[e][:],
                                                 start=(e == 0), stop=(e == NE - 1))
                            sl = dstT[h][:, ts(sc, 512)]
                            # RoPE: sl = stage*cos + swap(stage)*sin
                            stg = pa2.tile([128, 512], f32r, tag="stage")
                            nc.scalar.copy(stg[:], ps[:])
                            swp = pa2.tile([128, 512], f32r, tag="swap")
                            nc.sync.dma_start(out=swp[0:64, :], in_=stg[64:128, :])
                            nc.sync.dma_start(out=swp[64:128, :], in_=stg[0:64, :])
                            nc.vector.tensor_mul(swp[:], swp[:], sinT[:, ts(sc, 512)])
                            nc.vector.tensor_mul(sl, stg[:], cosT[:, ts(sc, 512)])
                            nc.vector.tensor_add(sl, sl, swp[:])
                    # v projection for this chunk's 4 token tiles
                    for t4 in range(4):
                        tt = sc * 4 + t4
                        ps = psA.tile([128, DG], f32, tag="pp", name="pp")
                        for e in range(NE):
                            nc.tensor.matmul(ps[:], xTs[e][:, ts(t4, 128)], wvs[e][:],
                                             start=(e == 0), stop=(e == NE - 1))
                        nc.scalar.copy(v_sb[tt][:], ps[:])

            # ---------------- Phase B: attention (transposed scores) + out-proj ----------------
            with tc.tile_pool(name="phB", bufs=1) as pb, \
                 tc.tile_pool(name="phB2", bufs=2) as pb2, \
                 tc.tile_pool(name="phB3", bufs=3) as pb3, \
                 tc.tile_pool(name="psS", bufs=3, space="PSUM") as psS, \
                 tc.tile_pool(name="psC", bufs=2, space="PSUM") as psC, \
                 tc.tile_pool(name="psZ", bufs=1, space="PSUM") as psZ, \
                 tc.tile_pool(name="psO", bufs=2, space="PSUM") as psO:
                # wo resident in SBUF for the whole phase (loads overlap qc=0)
                wo_sb = [pb.tile([128, E], f32r, tag=f"wosb{h}", name=f"wosb{h}")
                         for h in range(HG)]
                for h in range(HG):
                    nc.sync.dma_start(out=wo_sb[h][:],
                                      in_=wo_in[ts(h, 128), :].bitcast(f32r))

                for qc in range(NQC):
                    nkt = 4 * qc + 4        # k tiles needed for this q-chunk
                    q0 = qc * 512
                    ctxT = {}
                    for h in range(HG):
                        def issue_scores(kt):
                            """scores^T chunk [k-tile kt, q-chunk qc] -> exp -> bf16 aT."""
                            j = kt - 4 * qc   # >= 0 means diagonal straddle
                            c0 = max(0, j * 128)
                            w = 512 - c0
                            ps = psS.tile([128, 512], f32, tag="ps", name="ps")
                            nc.tensor.matmul(ps[:, ds(c0, w)], kT[h][:, ts(kt, 128)],
                                             qT[h][:, ds(q0 + c0, w)],
                                             start=True, stop=True)
                            at = pb3.tile([128, 512], bf16, tag="at", name="at")
                            nc.scalar.activation(at[:, ds(c0, w)], ps[:, ds(c0, w)],
                                                 mybir.ActivationFunctionType.Exp)
                            if j >= 0:
                                # causal: zero the strictly-lower triangle
                                # (q < k) of the diagonal 128x128 block
                                nc.gpsimd.affine_select(
                                    out=at[:, ds(c0, 128)], in_=at[:, ds(c0, 128)],
                                    compare_op=mybir.AluOpType.is_ge,
                                    fill=0.0, base=0,
                                    pattern=[[1, 128]], channel_multiplier=-1)
                            return at, c0, w

                        pc = psC.tile([128, 512], f32, tag="pc", name="pc")
                        pz = psZ.tile([128, 512], f32, tag="pz", name="pz")
                        pending = [issue_scores(0)]
                        if nkt > 1:
                            pending.append(issue_scores(1))
                        for kt in range(nkt):
                            if kt + 2 < nkt:
                                pending.append(issue_scores(kt + 2))
                            at, c0, w = pending[kt]
                            nc.tensor.matmul(pc[:, ds(c0, w)], v_sb[kt][:, ts(h, 128)],
                                             at[:, ds(c0, w)],
                                             start=(kt == 0), stop=(kt == nkt - 1),
                                             skip_group_check=True)
                            nc.tensor.matmul(pz[:, ds(c0, w)], ones_b[:],
                                             at[:, ds(c0, w)],
                                             start=(kt == 0), stop=(kt == nkt - 1),
                                             skip_group_check=True)
                        # normalize: ctxT = pc * (1/pz)
                        rz = pb2.tile([128, 512], f32, tag="rz", name="rz")
                        nc.vector.reciprocal_approx_fast(out=rz[:], in_=pz[:])
                        ct = pb2.tile([128, 512], f32r, tag=f"ctxT{h}", name=f"ctxT{h}")
                        nc.vector.tensor_mul(ct[:], pc[:], rz[:])
                        ctxT[h] = ct

                    # --- output projection for this q-chunk ---
                    for e4 in range(4):
                        for t4 in range(4):
                            row0 = qc * 512 + t4 * 128
                            po = psO.tile([128, 512], f32, tag="po", name="po")
                            for h in range(HG):
                                nc.tensor.matmul(po[:], ctxT[h][:, ts(t4, 128)],
                                                 wo_sb[h][:, ts(e4, 512)],
                                                 start=(h == 0), stop=(h == HG - 1))
                            ob = pb3.tile([128, 512], f32, tag="ob", name="ob", bufs=2)
                            nc.vector.tensor_copy(ob[:], po[:])
                            nc.sync.dma_start(out=out_d[row0:row0 + 128, ts(e4, 512)],
                                              in_=ob[:])
    nc.finalize()
    return nc


def _host_tables():
    half = D // 2
    inv = 1.0 / (ROPE_BASE ** (np.arange(half, dtype=np.float64) * 2.0 / D))
    ang = np.arange(S, dtype=np.float64)[None, :] * inv[:, None]   # [64, S]
    cos = np.cos(ang).astype(np.float32)
    sin = np.sin(ang).astype(np.float32)
    cosT = np.concatenate([cos, cos], axis=0)                      # [128, S]
    sinT = np.concatenate([-sin, sin], axis=0)                     # [128, S]
    return cosT, sinT


def kernel(x, start_pos, Wq, Wk, Wv, Wo):
    x = np.asarray(x, dtype=np.float32)
    Wq = np.asarray(Wq, dtype=np.float32)
    Wk = np.asarray(Wk, dtype=np.float32)
    Wv = np.asarray(Wv, dtype=np.float32)
    Wo = np.asarray(Wo, dtype=np.float32)
    B = x.shape[0]
    assert x.shape == (B, S, E) and B == 2

    cosT, sinT = _host_tables()
    perm = np.concatenate([np.arange(0, D, 2), np.arange(1, D, 2)])
    scale = 1.0 / np.sqrt(D)
    bf = ml_dtypes.bfloat16

    in_maps = []
    for c in range(8):
        b, g = c // 4, c % 4
        cols = slice(DG * g, DG * g + DG)
        wq = (Wq[:, cols] * scale).astype(np.float32).reshape(E, HG, D)[:, :, perm].reshape(E, DG)
        wk = Wk[:, cols].reshape(E, HG, D)[:, :, perm].reshape(E, DG)
        wv = Wv[:, cols]
        wo = np.ascontiguousarray(Wo[cols, :])
        in_maps.append({
            "xt": np.ascontiguousarray(x[b].T.astype(bf)),
            "wq": np.ascontiguousarray(wq.astype(bf)),
            "wk": np.ascontiguousarray(wk.astype(bf)),
            "wv": np.ascontiguousarray(wv.astype(bf)),
            "wo": wo,
            "cosT": cosT,
            "sinT": sinT,
        })

    if "nc" not in _CACHE:
        _CACHE["nc"] = build()
    nc = _CACHE["nc"]
    _CACHE["in_maps"] = in_maps
    res = run_bass_kernel_spmd(nc, in_maps, list(range(8)))
    parts = [res.results[c]["out"] for c in range(8)]
    out = np.stack([
        parts[0] + parts[1] + parts[2] + parts[3],
        parts[4] + parts[5] + parts[6] + parts[7],
    ]).astype(np.float32)
    return out


# revision 12
# speedup vs baseline: 1.6333x; 1.0763x over previous
"""Multi-head self-attention prefill (B=2, S=2048, E=2048, H=16, D=128) on 8 trn2 cores.

Sharding: core c -> batch b = c//4, head-group g = c%4 (heads 4g..4g+3).
Each core computes q/k/v projections for its 4 heads (column shard of Wq/Wk/Wv),
causal attention with RoPE, and a partial output projection (row shard of Wo).
Host sums the 4 partials per batch (all-reduce equivalent) and stacks batches.

Phase A: x arrives already transposed (and bf16-cast) from the host as xT[E,S];
it streams in sequence-chunk-major order so the first projection group starts
after ~2MB. Wq/Wk/Wv (bf16, host-cast) are SBUF-resident (24KB/partition) and
each is read exactly once from DRAM. RoPE runs in f32; qT/kT stay f32r.

Phase B: attention with transposed scores: s^T[k, q] = kT-tile^T @ qT, so the
exp() output (bf16) is directly the moving operand of the ctx matmul
ctx^T[d, q] = v^T @ exp(s^T) -- no PE transposes and no psum->sbuf copies in
the softmax chain. Causal masking zeroes the exp output's lower triangle on the
(otherwise idle) gpsimd engine. The softmax denominator z[q] comes from a
ones-stationary matmul over the same exp tiles (landing broadcast across
partitions); the 1/z scale is applied once to ctx^T per (head, q-chunk).
"""
import sys
sys.path.insert(0, "/opt/trn_rl_repo")
import numpy as np
import ml_dtypes

import concourse.bass as bass
import concourse.mybir as mybir
import concourse.tile as tile
from concourse import bacc
from concourse.bass import ds, ts
from concourse.bass_utils import run_bass_kernel_spmd

S = 2048          # sequence length (per batch)
E = 2048          # embedding dim
H = 16            # total heads
D = 128           # head dim
HG = 4            # heads per core
DG = HG * D       # 512: per-core projection width
NE = E // 128     # 16 contraction chunks
NTT = S // 128    # 16 token tiles of 128
NSC = 4           # sequence chunks of 512
NQC = 4           # q-chunks of 512
ROPE_BASE = 10000.0

f32 = mybir.dt.float32
f32r = mybir.dt.float32r
bf16 = mybir.dt.bfloat16

_CACHE = {}


def build():
    nc = bacc.Bacc(None)
    xt_in = nc.dram_tensor("xt", [E, S], bf16, kind="ExternalInput")
    wq_in = nc.dram_tensor("wq", [E, DG], bf16, kind="ExternalInput")
    wk_in = nc.dram_tensor("wk", [E, DG], bf16, kind="ExternalInput")
    wv_in = nc.dram_tensor("wv", [E, DG], bf16, kind="ExternalInput")
    wo_in = nc.dram_tensor("wo", [DG, E], f32, kind="ExternalInput")
    cos_in = nc.dram_tensor("cosT", [128, S], f32, kind="ExternalInput")
    sin_in = nc.dram_tensor("sinT", [128, S], f32, kind="ExternalInput")
    out_d = nc.dram_tensor("out", [S, E], f32, kind="ExternalOutput")

    with tile.TileContext(nc) as tc:
        with tc.tile_pool(name="persist", bufs=1) as pp:
            # persistent across phases
            qT = [pp.tile([128, S], f32r, tag=f"qT{h}", name=f"qT{h}") for h in range(HG)]
            kT = [pp.tile([128, S], f32r, tag=f"kT{h}", name=f"kT{h}") for h in range(HG)]
            v_sb = [pp.tile([128, DG], bf16, tag=f"v{tt}", name=f"v{tt}") for tt in range(NTT)]
            ones_b = pp.tile([128, 128], bf16, tag="ones_b")
            nc.gpsimd.memset(ones_b[:], 1.0)

            # ---------------- Phase A: projections + RoPE ----------------
            with tc.tile_pool(name="phA", bufs=1) as pa, \
                 tc.tile_pool(name="phA2", bufs=2) as pa2, \
                 tc.tile_pool(name="psA", bufs=2, space="PSUM") as psA:
                def load_x_chunk(sc):
                    """x^T tiles for one 512-token sequence chunk (bufs=2 so the
                    next chunk's loads prefetch during this chunk's compute)."""
                    tiles = []
                    for e in range(NE):
                        xt = pa2.tile([128, 512], bf16, tag=f"xT{e}", name=f"xT{e}")
                        nc.sync.dma_start(out=xt[:],
                                          in_=xt_in[ts(e, 128), ts(sc, 512)])
                        tiles.append(xt)
                    return tiles

                # first x chunk and wq interleaved pairwise (the first q group
                # consumes both e-ascending); wk next (k groups start ~14us in);
                # tables after that (RoPE muls tolerate late tables -- nothing
                # in phase A reads qT/kT); wv last before the x prefetch
                xcur = []
                wqs, wks, wvs = [], [], []
                for tag, w_in, lst in (("wq", wq_in, wqs), ("wk", wk_in, wks),
                                       ("wv", wv_in, wvs)):
                    for e in range(NE):
                        lst.append(pa.tile([128, DG], bf16, tag=f"{tag}{e}",
                                           name=f"{tag}{e}"))
                for e in range(NE):
                    xt = pa2.tile([128, 512], bf16, tag=f"xT{e}", name=f"xT{e}")
                    nc.sync.dma_start(out=xt[:], in_=xt_in[ts(e, 128), ds(0, 512)])
                    xcur.append(xt)
                    nc.sync.dma_start(out=wqs[e][:], in_=wq_in[ts(e, 128), :])
                for e in range(NE):
                    nc.sync.dma_start(out=wks[e][:], in_=wk_in[ts(e, 128), :])
                cosT = pa.tile([128, S], f32r, tag="cos")
                nc.sync.dma_start(out=cosT[:], in_=cos_in[:].bitcast(f32r))
                sinT = pa.tile([128, S], f32r, tag="sin")
                nc.sync.dma_start(out=sinT[:], in_=sin_in[:].bitcast(f32r))
                for e in range(NE):
                    nc.sync.dma_start(out=wvs[e][:], in_=wv_in[ts(e, 128), :])

                for sc in range(NSC):
                    xTs = xcur
                    if sc + 1 < NSC:
                        xcur = load_x_chunk(sc + 1)
                    # q/k projections for this sequence chunk
                    for wts, dstT in ((wqs, qT), (wks, kT)):
                        for h in range(HG):
                            ps = psA.tile([128, 512], f32, tag="pp", name="pp")
                            for e in range(NE):
                                nc.tensor.matmul(ps[:], wts[e][:, ts(h, 128)],
                                                 xTs[e][:],
                                                 start=(e == 0), stop=(e == NE - 1))
                            sl = dstT[h][:, ts(sc, 512)]
                            # RoPE: sl = stage*cos + swap(stage)*sin
                            # (swaps ride the Act HWDGE queue so bulk loads on
                            # the Sync queue never block the stage rotation)
                            stg = pa2.tile([128, 512], f32r, tag="stage", bufs=4)
                            nc.scalar.copy(stg[:], ps[:])
                            swp = pa2.tile([128, 512], f32r, tag="swap", bufs=4)
                            nc.scalar.dma_start(out=swp[0:64, :], in_=stg[64:128, :])
                            nc.scalar.dma_start(out=swp[64:128, :], in_=stg[0:64, :])
                            nc.vector.tensor_mul(swp[:], swp[:], sinT[:, ts(sc, 512)])
                            nc.vector.tensor_mul(sl, stg[:], cosT[:, ts(sc, 512)])
                            nc.vector.tensor_add(sl, sl, swp[:])
                    # v projection for this chunk's 4 token tiles
                    for t4 in range(4):
                        tt = sc * 4 + t4
                        ps = psA.tile([128, DG], f32, tag="pp", name="pp")
                        for e in range(NE):
                            nc.tensor.matmul(ps[:], xTs[e][:, ts(t4, 128)], wvs[e][:],
                                             start=(e == 0), stop=(e == NE - 1))
                        nc.scalar.copy(v_sb[tt][:], ps[:])

            # ---------------- Phase B: attention (transposed scores) + out-proj ----------------
            with tc.tile_pool(name="phB", bufs=1) as pb, \
                 tc.tile_pool(name="phB2", bufs=2) as pb2, \
                 tc.tile_pool(name="phB3", bufs=3) as pb3, \
                 tc.tile_pool(name="psS", bufs=3, space="PSUM") as psS, \
                 tc.tile_pool(name="psC", bufs=2, space="PSUM") as psC, \
                 tc.tile_pool(name="psZ", bufs=1, space="PSUM") as psZ, \
                 tc.tile_pool(name="psO", bufs=2, space="PSUM") as psO:
                # wo resident in SBUF for the whole phase (loads overlap qc=0)
                wo_sb = [pb.tile([128, E], f32r, tag=f"wosb{h}", name=f"wosb{h}")
                         for h in range(HG)]
                for h in range(HG):
                    nc.sync.dma_start(out=wo_sb[h][:],
                                      in_=wo_in[ts(h, 128), :].bitcast(f32r))

                for qc in range(NQC):
                    nkt = 4 * qc + 4        # k tiles needed for this q-chunk
                    q0 = qc * 512
                    ctxT = {}
                    for h in range(HG):
                        def issue_scores(kt):
                            """scores^T chunk [k-tile kt, q-chunk qc] -> exp -> bf16 aT."""
                            j = kt - 4 * qc   # >= 0 means diagonal straddle
                            c0 = max(0, j * 128)
                            w = 512 - c0
                            ps = psS.tile([128, 512], f32, tag="ps", name="ps")
                            nc.tensor.matmul(ps[:, ds(c0, w)], kT[h][:, ts(kt, 128)],
                                             qT[h][:, ds(q0 + c0, w)],
                                             start=True, stop=True)
                            at = pb3.tile([128, 512], bf16, tag="at", name="at")
                            nc.scalar.activation(at[:, ds(c0, w)], ps[:, ds(c0, w)],
                                                 mybir.ActivationFunctionType.Exp)
                            if j >= 0:
                                # causal: zero the strictly-lower triangle
                                # (q < k) of the diagonal 128x128 block
                                nc.gpsimd.affine_select(
                                    out=at[:, ds(c0, 128)], in_=at[:, ds(c0, 128)],
                                    compare_op=mybir.AluOpType.is_ge,
                                    fill=0.0, base=0,
                                    pattern=[[1, 128]], channel_multiplier=-1)
                            return at, c0, w

                        pc = psC.tile([128, 512], f32, tag="pc", name="pc")
                        pz = psZ.tile([128, 512], f32, tag="pz", name="pz")
                        pending = [issue_scores(0)]
                        if nkt > 1:
                            pending.append(issue_scores(1))
                        for kt in range(nkt):
                            if kt + 2 < nkt:
                                pending.append(issue_scores(kt + 2))
                            at, c0, w = pending[kt]
                            nc.tensor.matmul(pc[:, ds(c0, w)], v_sb[kt][:, ts(h, 128)],
                                             at[:, ds(c0, w)],
                                             start=(kt == 0), stop=(kt == nkt - 1),
                                             skip_group_check=True)
                            nc.tensor.matmul(pz[:, ds(c0, w)], ones_b[:],
                                             at[:, ds(c0, w)],
                                             start=(kt == 0), stop=(kt == nkt - 1),
                                             skip_group_check=True)
                        # normalize: ctxT = pc * (1/pz)
                        rz = pb2.tile([128, 512], f32, tag="rz", name="rz")
                        nc.vector.reciprocal_approx_fast(out=rz[:], in_=pz[:])
                        ct = pb2.tile([128, 512], f32r, tag=f"ctxT{h}", name=f"ctxT{h}")
                        nc.vector.tensor_mul(ct[:], pc[:], rz[:])
                        ctxT[h] = ct

                    # --- output projection for this q-chunk ---
                    for e4 in range(4):
                        for t4 in range(4):
                            row0 = qc * 512 + t4 * 128
                            po = psO.tile([128, 512], f32, tag="po", name="po")
                            for h in range(HG):
                                nc.tensor.matmul(po[:], ctxT[h][:, ts(t4, 128)],
                                                 wo_sb[h][:, ts(e4, 512)],
                                                 start=(h == 0), stop=(h == HG - 1))
                            ob = pb3.tile([128, 512], f32, tag="ob", name="ob", bufs=2)
                            nc.vector.tensor_copy(ob[:], po[:])
                            nc.sync.dma_start(out=out_d[row0:row0 + 128, ts(e4, 512)],
                                              in_=ob[:])
    nc.finalize()
    return nc


def _host_tables():
    half = D // 2
    inv = 1.0 / (ROPE_BASE ** (np.arange(half, dtype=np.float64) * 2.0 / D))
    ang = np.arange(S, dtype=np.float64)[None, :] * inv[:, None]   # [64, S]
    cos = np.cos(ang).astype(np.float32)
    sin = np.sin(ang).astype(np.float32)
    cosT = np.concatenate([cos, cos], axis=0)                      # [128, S]
    sinT = np.concatenate([-sin, sin], axis=0)                     # [128, S]
    return cosT, sinT


def kernel(x, start_pos, Wq, Wk, Wv, Wo):
    x = np.asarray(x, dtype=np.float32)
    Wq = np.asarray(Wq, dtype=np.float32)
    Wk = np.asarray(Wk, dtype=np.float32)
    Wv = np.asarray(Wv, dtype=np.float32)
    Wo = np.asarray(Wo, dtype=np.float32)
    B = x.shape[0]
    assert x.shape == (B, S, E) and B == 2

    cosT, sinT = _host_tables()
    perm = np.concatenate([np.arange(0, D, 2), np.arange(1, D, 2)])
    scale = 1.0 / np.sqrt(D)
    bf = ml_dtypes.bfloat16

    in_maps = []
    for c in range(8):
        b, g = c // 4, c % 4
        cols = slice(DG * g, DG * g + DG)
        wq = (Wq[:, cols] * scale).astype(np.float32).reshape(E, HG, D)[:, :, perm].reshape(E, DG)
        wk = Wk[:, cols].reshape(E, HG, D)[:, :, perm].reshape(E, DG)
        wv = Wv[:, cols]
        wo = np.ascontiguousarray(Wo[cols, :])
        in_maps.append({
            "xt": np.ascontiguousarray(x[b].T.astype(bf)),
            "wq": np.ascontiguousarray(wq.astype(bf)),
            "wk": np.ascontiguousarray(wk.astype(bf)),
            "wv": np.ascontiguousarray(wv.astype(bf)),
            "wo": wo,
            "cosT": cosT,
            "sinT": sinT,
        })

    if "nc" not in _CACHE:
        _CACHE["nc"] = build()
    nc = _CACHE["nc"]
    _CACHE["in_maps"] = in_maps
    res = run_bass_kernel_spmd(nc, in_maps, list(range(8)))
    parts = [res.results[c]["out"] for c in range(8)]
    out = np.stack([
        parts[0] + parts[1] + parts[2] + parts[3],
        parts[4] + parts[5] + parts[6] + parts[7],
    ]).astype(np.float32)
    return out
